# revision 28
# baseline (speedup 1.0000x reference)
"""Trainium2 Bass kernel for adaptive_high_order_residual_v2 (ORDER=2 masked
sign-binarization, per-row stats).

Full-input contract: kernel(x, mask) takes the complete (4096, 11008) arrays,
shards rows across 8 NeuronCores (512 rows each; per-row reductions make this
embarrassingly parallel), runs one SPMD Bass program, and concatenates the
per-core outputs.

Math per row (exact restructuring of the reference, ORDER = 2):
  T     = x*m                                   (masked input; in-place on x)
  cnt   = sum(m); r1 = sum(T); r2 = sum(T^2)
  mean1 = r1/cnt ; var1 = r2/cnt - mean1^2 ; s1 = sqrt(var1 * 2/pi)
  ab    = |T - mean1|  (+ accum; invalid entries contribute |mean1| each,
                        corrected by scalar algebra)
  b1    = sign(T - mean1)      (unmasked: invalid entries give sign(-mean1))
  q     = (ab - s1) * b1       (+ accum sum q; invalid garbage is the
                                per-row constant s1*sign(mean1)-mean1,
                                corrected by scalar algebra)
  sum q^2 = (r2 - mean1*r1) + cnt*s1^2 - 2*s1*sum|T - mean1|_masked
            (no elementwise pass needed)
  mean2, s2 from the corrected sums; K = mean1 + mean2
  out   = (K + s1*b1 + s2*b2) * m,  b2 = sign(q - mean2)

Engine split (measured per-chunk-of-2752 on HW):
  ACT : mask cast + cnt, sum(T^2) (3/4 chunks), abs+accum, sign1, sign2
  DVE : T=x*m+r1 (STT), sum(T^2) (1/4), q (STT), u=s2*b2+K (TS 4x),
        bs1=s1*b1 (TS 4x), w=u+bs1 (TT 2x), out=w*m (TT)
  Pool: unused - concurrent GPSIMD activity degrades DVE 2x/4x modes ~2.5x
        (SBUF port contention), a net loss every time it was tried.

Emission is software-pipelined (A(0) B(0) A(1) C(0) B(1) A(2) C(1) ...) so
the tile list-scheduler interleaves next-block stage-A work into the C-stage
stream; per-block scalar reductions run under high_priority so they jump both
engine queues.
"""

import sys

import numpy as np

sys.path.insert(0, "/opt/trn_rl_repo")

R = 512          # rows per core
N = 11008        # columns
P = 128          # SBUF partitions per row-block
NBLK = R // P    # 4 blocks per core
CW = 1376        # column chunk width
NCH = N // CW    # 4 chunks per block
NCORES = 8
C2 = 0.6366197723675814  # 2/pi
EPS = 1e-30

# Per-chunk engine assignment for the flexible accumulation passes
# ("act" or "dve"); index is chunk id within a block.
SQ_ENGINE = ["act"] * 6 + ["dve"] * 2       # sum(T^2) accum
AB_ENGINE = ["act"] * NCH                      # |T - mean1| + accum

_CACHE = {}


def _build_program():
    import concourse.bacc as bacc
    import concourse.mybir as mybir
    from concourse.tile import TileContext

    F32 = mybir.dt.float32
    F16 = mybir.dt.float16
    U8 = mybir.dt.uint8
    Alu = mybir.AluOpType
    Act = mybir.ActivationFunctionType

    nc = bacc.Bacc()
    x = nc.dram_tensor("x", [R, N], F32, kind="ExternalInput")
    mk = nc.dram_tensor("mask", [R, N], U8, kind="ExternalInput")
    out = nc.dram_tensor("out", [R, N], F32, kind="ExternalOutput")

    H = CW // 2

    def pieces_for(split_first):
        ps = []
        for c in range(NCH):
            if c == 0 and split_first:
                ps.append((c, 0, H))
                ps.append((c, H, H))
            else:
                ps.append((c, 0, CW))
        return ps

    with TileContext(nc) as tc:
        with (
            tc.tile_pool(name="xq", bufs=2 * NCH + 2) as xq_pool,   # x -> T -> q -> out
            tc.tile_pool(name="m8", bufs=2 * NCH) as m8_pool,   # u8 mask
            tc.tile_pool(name="b1", bufs=2 * NCH - 1) as b1_pool,   # sign1 -> s1*b1 (f16)
            tc.tile_pool(name="ab", bufs=3) as ab_pool,         # |T - mean1| (f32)
            tc.tile_pool(name="b2", bufs=3) as b2_pool,         # sign2 -> u -> w (f16)
            tc.tile_pool(name="t16", bufs=3) as t16_pool,       # discard outputs (f16)
            tc.tile_pool(name="sc", bufs=2) as sc_pool,         # accums + scalars
        ):
            S = [dict() for _ in range(NBLK)]

            def col(t, i):
                return t[:, i : i + 1]

            def reduce_pieces(dst, acc, k, npieces, red):
                # piece-major accum columns: col = p*k + q
                if npieces in (8, 9):
                    nc.vector.tensor_add(
                        red[:, 0 : 4 * k], acc[:, 0 : 4 * k], acc[:, 4 * k : 8 * k]
                    )
                    nc.vector.tensor_add(
                        red[:, 4 * k : 6 * k], red[:, 0 : 2 * k],
                        red[:, 2 * k : 4 * k],
                    )
                    if npieces == 8:
                        nc.vector.tensor_add(
                            dst, red[:, 4 * k : 5 * k], red[:, 5 * k : 6 * k]
                        )
                    else:
                        nc.vector.tensor_add(
                            red[:, 6 * k : 7 * k], red[:, 4 * k : 5 * k],
                            red[:, 5 * k : 6 * k],
                        )
                        nc.vector.tensor_add(
                            dst, red[:, 6 * k : 7 * k], acc[:, 8 * k : 9 * k]
                        )
                elif npieces == 4:
                    nc.vector.tensor_add(
                        red[:, 0 : 2 * k], acc[:, 0 : 2 * k], acc[:, 2 * k : 4 * k]
                    )
                    nc.vector.tensor_add(dst, red[:, 0:k], red[:, k : 2 * k])
                elif npieces == 5:
                    nc.vector.tensor_add(
                        red[:, 0 : 2 * k], acc[:, 0 : 2 * k], acc[:, 2 * k : 4 * k]
                    )
                    nc.vector.tensor_add(
                        red[:, 2 * k : 3 * k], red[:, 0:k], red[:, k : 2 * k]
                    )
                    nc.vector.tensor_add(
                        dst, red[:, 2 * k : 3 * k], acc[:, 4 * k : 5 * k]
                    )
                else:
                    raise AssertionError(npieces)

            def emit_A(b):
                s = S[b]
                r0 = b * P
                s["xt"] = [
                    xq_pool.tile([P, CW], F32, name=f"xt{b}_{c}", tag="xq")
                    for c in range(NCH)
                ]
                s["mt"] = [
                    m8_pool.tile([P, CW], U8, name=f"mt{b}_{c}", tag="m8")
                    for c in range(NCH)
                ]
                s["b1"] = [
                    b1_pool.tile([P, CW], F16, name=f"b1_{b}_{c}", tag="b1")
                    for c in range(NCH)
                ]
                s["accA"] = sc_pool.tile([P, 27], F32, name=f"accA_{b}", tag="accA")
                s["accB"] = sc_pool.tile([P, 18], F32, name=f"accB_{b}", tag="accB")
                s["redA"] = sc_pool.tile([P, 21], F32, name=f"redA_{b}", tag="redA")
                s["redB"] = sc_pool.tile([P, 14], F32, name=f"redB_{b}", tag="redB")
                s["stA"] = sc_pool.tile([P, 3], F32, name=f"stA_{b}", tag="stA")
                s["stB"] = sc_pool.tile([P, 2], F32, name=f"stB_{b}", tag="stB")
                s["sv"] = sc_pool.tile([P, 26], F32, name=f"sv_{b}", tag="sv")
                sv = s["sv"]
                for nmi, i in (
                    ("cntc", 0), ("inv", 1), ("mean1", 2), ("nm1", 3),
                    ("e1", 4), ("v1c", 5), ("s1", 6), ("ns1", 7),
                    ("sg1", 8), ("am1", 9), ("ncnt", 10), ("nqinv", 11),
                    ("sqc", 12), ("mean2", 13), ("nm2", 14), ("nam1", 15),
                    ("sTm", 16), ("t1", 17), ("t2", 18), ("ns1x2", 19),
                    ("sq2", 20), ("e2", 21), ("v2c", 22), ("s2", 23),
                    ("kk", 24),
                ):
                    s[nmi] = col(sv, i)

                xt, mt, accA = s["xt"], s["mt"], s["accA"]
                pa = pieces_for(b == 0)
                s["pa"] = pa
                for i, (c, o, wd) in enumerate(pa):
                    nc.sync.dma_start(
                        xt[c][:, o : o + wd],
                        x[r0 : r0 + P, c * CW + o : c * CW + o + wd],
                    )
                    nc.sync.dma_start(
                        mt[c][:, o : o + wd],
                        mk[r0 : r0 + P, c * CW + o : c * CW + o + wd],
                    )
                    # mask cast (value discarded) + cnt partial
                    cnt_t = t16_pool.tile([P, wd], F16, name=f"cnt{b}_{i}", tag="t16")
                    nc.scalar.activation(
                        cnt_t[:], mt[c][:, o : o + wd], Act.Copy,
                        accum_out=col(accA, i * 3 + 0),
                    )
                    # T = x*m (in place) + r1 partial
                    nc.vector.scalar_tensor_tensor(
                        xt[c][:, o : o + wd], xt[c][:, o : o + wd], 1.0,
                        mt[c][:, o : o + wd],
                        Alu.bypass, Alu.mult,
                        accum_out=col(accA, i * 3 + 1),
                    )
                    # r2 partial: sum(T^2); output value discarded
                    sq_t = t16_pool.tile([P, wd], F16, name=f"sq{b}_{i}", tag="t16")
                    if SQ_ENGINE[c] == "act":
                        nc.scalar.activation(
                            sq_t[:], xt[c][:, o : o + wd], Act.Square,
                            accum_out=col(accA, i * 3 + 2),
                        )
                    else:
                        nc.vector.scalar_tensor_tensor(
                            sq_t[:], xt[c][:, o : o + wd], 1.0,
                            xt[c][:, o : o + wd],
                            Alu.bypass, Alu.mult,
                            accum_out=col(accA, i * 3 + 2),
                        )

                # first-order stats; high priority so they jump both queues
                with tc.high_priority():
                    reduce_pieces(s["stA"][:, 0:3], accA, 3, len(pa), s["redA"])
                    cnt, r1, r2 = (col(s["stA"], i) for i in range(3))
                    s["cnt"], s["r1"], s["r2"] = cnt, r1, r2
                    nc.vector.tensor_scalar(s["cntc"], cnt, 1.0, None, Alu.max)
                    nc.vector.reciprocal(s["inv"], s["cntc"])
                    nc.vector.tensor_mul(s["mean1"], r1, s["inv"])
                    nc.vector.tensor_scalar(s["nm1"], s["mean1"], -1.0, None, Alu.mult)
                    nc.vector.tensor_mul(s["e1"], r2, s["inv"])
                    # v1c = max((e1 - mean1^2) * C2, EPS)
                    nc.vector.scalar_tensor_tensor(
                        s["v1c"], s["mean1"], s["nm1"], s["e1"], Alu.mult, Alu.add
                    )
                    nc.vector.tensor_scalar(
                        s["v1c"], s["v1c"], C2, EPS, Alu.mult, Alu.max
                    )
                    nc.scalar.activation(s["s1"], s["v1c"], Act.Sqrt)
                    nc.vector.tensor_scalar(s["ns1"], s["s1"], -1.0, None, Alu.mult)

            def emit_B(b):
                s = S[b]
                xt, mt, b1, accB = s["xt"], s["mt"], s["b1"], s["accB"]
                nm1, ns1 = s["nm1"], s["ns1"]
                pb = pieces_for(True)
                s["pb"] = pb
                for i, (c, o, wd) in enumerate(pb):
                    ab = ab_pool.tile([P, wd], F32, name=f"ab{b}_{i}", tag="ab")
                    if AB_ENGINE[c] == "act":
                        nc.scalar.activation(
                            ab[:], xt[c][:, o : o + wd], Act.Abs, bias=nm1,
                            accum_out=col(accB, i * 2 + 0),
                        )
                    else:
                        # |T + nm1| = abs_max(T + nm1, 0)
                        nc.vector.tensor_scalar(
                            ab[:], xt[c][:, o : o + wd], nm1, 0.0,
                            Alu.add, Alu.abs_max,
                            accum_out=col(accB, i * 2 + 0),
                        )
                    nc.scalar.activation(
                        b1[c][:, o : o + wd], xt[c][:, o : o + wd],
                        Act.Sign, bias=nm1,
                    )
                    # q = (ab - s1) * b1, in place onto the T tile
                    nc.vector.scalar_tensor_tensor(
                        xt[c][:, o : o + wd], ab[:], ns1, b1[c][:, o : o + wd],
                        Alu.add, Alu.mult,
                        accum_out=col(accB, i * 2 + 1),
                    )

                # correction scalars (low priority; only needed at the reduce)
                nc.scalar.activation(s["sg1"], s["mean1"], Act.Sign)
                nc.vector.tensor_mul(s["am1"], s["mean1"], s["sg1"])
                nc.vector.tensor_scalar(
                    s["ncnt"], s["cnt"], -1.0, float(N), Alu.mult, Alu.add
                )
                nc.vector.scalar_tensor_tensor(
                    s["nqinv"], s["sg1"], s["ns1"], s["mean1"], Alu.mult, Alu.add
                )
                nc.vector.tensor_scalar(s["nam1"], s["am1"], -1.0, None, Alu.mult)

                # second-order stats
                with tc.high_priority():
                    reduce_pieces(s["stB"][:, 0:2], accB, 2, len(pb), s["redB"])
                    sab, sq = col(s["stB"], 0), col(s["stB"], 1)
                    nc.vector.scalar_tensor_tensor(
                        s["sqc"], s["ncnt"], s["nqinv"], sq, Alu.mult, Alu.add
                    )
                    nc.vector.tensor_mul(s["mean2"], s["sqc"], s["inv"])
                    nc.vector.tensor_scalar(
                        s["nm2"], s["mean2"], -1.0, None, Alu.mult
                    )
                    nc.vector.scalar_tensor_tensor(
                        s["sTm"], s["ncnt"], s["nam1"], sab, Alu.mult, Alu.add
                    )
                    nc.vector.scalar_tensor_tensor(
                        s["t1"], s["r1"], s["nm1"], s["r2"], Alu.mult, Alu.add
                    )
                    nc.vector.scalar_tensor_tensor(
                        s["t2"], s["cnt"], s["v1c"], s["t1"], Alu.mult, Alu.add
                    )
                    nc.vector.tensor_scalar(s["ns1x2"], s["s1"], -2.0, None, Alu.mult)
                    nc.vector.scalar_tensor_tensor(
                        s["sq2"], s["sTm"], s["ns1x2"], s["t2"], Alu.mult, Alu.add
                    )
                    nc.vector.tensor_mul(s["e2"], s["sq2"], s["inv"])
                    nc.vector.scalar_tensor_tensor(
                        s["v2c"], s["mean2"], s["nm2"], s["e2"], Alu.mult, Alu.add
                    )
                    nc.vector.tensor_scalar(
                        s["v2c"], s["v2c"], C2, EPS, Alu.mult, Alu.max
                    )
                    nc.scalar.activation(s["s2"], s["v2c"], Act.Sqrt)
                    nc.vector.tensor_add(s["kk"], s["mean1"], s["mean2"])

            def emit_C(b):
                s = S[b]
                r0 = b * P
                xt, mt, b1 = s["xt"], s["mt"], s["b1"]
                nm2, s1, s2, kk = s["nm2"], s["s1"], s["s2"], s["kk"]
                u_on_act = False
                for i, (c, o, wd) in enumerate(pieces_for(True)):
                    b2 = b2_pool.tile([P, wd], F16, name=f"b2_{b}_{i}", tag="b2")
                    nc.scalar.activation(
                        b2[:], xt[c][:, o : o + wd], Act.Sign, bias=nm2
                    )
                    # u = s2*b2 + K (TS 4x, in place)
                    if u_on_act:
                        nc.scalar.activation(
                            b2[:], b2[:], Act.Identity, bias=kk, scale=s2
                        )
                    else:
                        nc.vector.tensor_scalar(
                            b2[:], b2[:], s2, kk, Alu.mult, Alu.add
                        )
                    # bs1 = s1*b1 (TS 4x, in place)
                    nc.vector.tensor_scalar(
                        b1[c][:, o : o + wd], b1[c][:, o : o + wd], s1, None,
                        Alu.mult,
                    )
                    # w = u + bs1 (TT 2x, in place)
                    nc.vector.tensor_add(b2[:], b2[:], b1[c][:, o : o + wd])
                    # out = w * m -> f32, overwrites the q tile
                    nc.vector.tensor_mul(
                        xt[c][:, o : o + wd], b2[:], mt[c][:, o : o + wd]
                    )
                    nc.sync.dma_start(
                        out[r0 : r0 + P, c * CW + o : c * CW + o + wd],
                        xt[c][:, o : o + wd],
                    )

            # software-pipelined emission: next-block A lands ahead of the
            # previous block's C in every engine queue
            emit_A(0)
            emit_B(0)
            for b in range(1, NBLK):
                emit_A(b)
                emit_C(b - 1)
                emit_B(b)
            emit_C(NBLK - 1)

    return nc


def get_program():
    if "nc" not in _CACHE:
        nc = _build_program()
        nc.finalize()
        _CACHE["nc"] = nc
    return _CACHE["nc"]


def kernel(x: np.ndarray, mask: np.ndarray) -> np.ndarray:
    import time

    from concourse.bass_utils import run_bass_kernel_spmd

    x = np.ascontiguousarray(np.asarray(x, dtype=np.float32))
    mask = np.ascontiguousarray(np.asarray(mask))
    if mask.dtype == np.bool_ or mask.dtype == np.uint8:
        mask_u8 = mask.view(np.uint8)
    else:
        mask_u8 = (mask != 0).astype(np.uint8)
    assert x.shape == (R * NCORES, N), x.shape
    assert mask_u8.shape == (R * NCORES, N), mask_u8.shape

    nc = get_program()
    in_maps = [
        {
            "x": x[k * R : (k + 1) * R],
            "mask": mask_u8[k * R : (k + 1) * R],
        }
        for k in range(NCORES)
    ]
    last_err = None
    for attempt in range(3):
        try:
            res = run_bass_kernel_spmd(nc, in_maps, core_ids=list(range(NCORES)))
            return np.concatenate([r["out"] for r in res.results], axis=0)
        except Exception as e:  # transient NRT/device hiccups
            last_err = e
            if attempt < 2:
                time.sleep(10)
    raise last_err


if __name__ == "__main__":
    xs = np.random.randn(R * NCORES, N).astype(np.float32)
    ms = (np.random.randint(0, 2, (R * NCORES, N))).astype(bool)
    y = kernel(xs, ms)
    print(y.shape, y.dtype)


# revision 29
# speedup vs baseline: 1.0296x; 1.0296x over previous
"""Trainium2 Bass kernel for adaptive_high_order_residual_v2 (ORDER=2 masked
sign-binarization, per-row stats).

Full-input contract: kernel(x, mask) takes the complete (4096, 11008) arrays,
shards rows across 8 NeuronCores (512 rows each; per-row reductions make this
embarrassingly parallel), runs one SPMD Bass program, and concatenates the
per-core outputs.

Math per row (exact restructuring of the reference, ORDER = 2):
  T     = x*m                                   (masked input; in-place on x)
  cnt   = sum(m); r1 = sum(T); r2 = sum(T^2)
  mean1 = r1/cnt ; var1 = r2/cnt - mean1^2 ; s1 = sqrt(var1 * 2/pi)
  ab    = |T - mean1|  (+ accum; invalid entries contribute |mean1| each,
                        corrected by scalar algebra)
  b1    = sign(T - mean1)      (unmasked: invalid entries give sign(-mean1))
  q     = (ab - s1) * b1       (+ accum sum q; invalid garbage is the
                                per-row constant s1*sign(mean1)-mean1,
                                corrected by scalar algebra)
  sum q^2 = (r2 - mean1*r1) + cnt*s1^2 - 2*s1*sum|T - mean1|_masked
            (no elementwise pass needed)
  mean2, s2 from the corrected sums; K = mean1 + mean2
  out   = (K + s1*b1 + s2*b2) * m,  b2 = sign(q - mean2)

Engine split (measured per-chunk-of-2752 on HW):
  ACT : mask cast + cnt, sum(T^2) (3/4 chunks), abs+accum, sign1, sign2
  DVE : T=x*m+r1 (STT), sum(T^2) (1/4), q (STT), u=s2*b2+K (TS 4x),
        bs1=s1*b1 (TS 4x), w=u+bs1 (TT 2x), out=w*m (TT)
  Pool: unused - concurrent GPSIMD activity degrades DVE 2x/4x modes ~2.5x
        (SBUF port contention), a net loss every time it was tried.

Emission is software-pipelined (A(0) B(0) A(1) C(0) B(1) A(2) C(1) ...) so
the tile list-scheduler interleaves next-block stage-A work into the C-stage
stream; per-block scalar reductions run under high_priority so they jump both
engine queues.
"""

import sys

import numpy as np

sys.path.insert(0, "/opt/trn_rl_repo")

R = 512          # rows per core
N = 11008        # columns
P = 128          # SBUF partitions per row-block
NBLK = R // P    # 4 blocks per core
CW = 2752        # column chunk width
NCH = N // CW    # 4 chunks per block
NCORES = 8
C2 = 0.6366197723675814  # 2/pi
EPS = 1e-30

# Per-chunk engine assignment for the flexible accumulation passes
# ("act" or "dve"); index is chunk id within a block.
SQ_ENGINE = ["act", "act", "act", "dve"]       # sum(T^2) accum
AB_ENGINE = ["act"] * NCH                      # |T - mean1| + accum

_CACHE = {}


def _build_program():
    import concourse.bacc as bacc
    import concourse.mybir as mybir
    from concourse.tile import TileContext

    F32 = mybir.dt.float32
    F16 = mybir.dt.float16
    U8 = mybir.dt.uint8
    Alu = mybir.AluOpType
    Act = mybir.ActivationFunctionType

    nc = bacc.Bacc()
    x = nc.dram_tensor("x", [R, N], F32, kind="ExternalInput")
    mk = nc.dram_tensor("mask", [R, N], U8, kind="ExternalInput")
    out = nc.dram_tensor("out", [R, N], F32, kind="ExternalOutput")

    H = CW // 2

    def pieces_for(split_first):
        ps = []
        for c in range(NCH):
            if c == 0 and split_first:
                ps.append((c, 0, H))
                ps.append((c, H, H))
            else:
                ps.append((c, 0, CW))
        return ps

    with TileContext(nc) as tc:
        with (
            tc.tile_pool(name="xq", bufs=2 * NCH + 1) as xq_pool,   # x -> T -> q -> out
            tc.tile_pool(name="m8", bufs=2 * NCH) as m8_pool,   # u8 mask
            tc.tile_pool(name="b1", bufs=2 * NCH - 1) as b1_pool,   # sign1 -> s1*b1 (f16)
            tc.tile_pool(name="ab", bufs=2) as ab_pool,         # |T - mean1| (f32)
            tc.tile_pool(name="b2", bufs=3) as b2_pool,         # sign2 -> u -> w (f16)
            tc.tile_pool(name="t16", bufs=2) as t16_pool,       # discard outputs (f16)
            tc.tile_pool(name="sc", bufs=2) as sc_pool,         # accums + scalars
        ):
            S = [dict() for _ in range(NBLK)]

            def col(t, i):
                return t[:, i : i + 1]

            def reduce_pieces(dst, acc, k, npieces, red):
                # piece-major accum columns: col = p*k + q
                if npieces == 4:
                    nc.vector.tensor_add(
                        red[:, 0 : 2 * k], acc[:, 0 : 2 * k], acc[:, 2 * k : 4 * k]
                    )
                    nc.vector.tensor_add(dst, red[:, 0:k], red[:, k : 2 * k])
                elif npieces == 5:
                    nc.vector.tensor_add(
                        red[:, 0 : 2 * k], acc[:, 0 : 2 * k], acc[:, 2 * k : 4 * k]
                    )
                    nc.vector.tensor_add(
                        red[:, 2 * k : 3 * k], red[:, 0:k], red[:, k : 2 * k]
                    )
                    nc.vector.tensor_add(
                        dst, red[:, 2 * k : 3 * k], acc[:, 4 * k : 5 * k]
                    )
                else:
                    raise AssertionError(npieces)

            def emit_A(b):
                s = S[b]
                r0 = b * P
                s["xt"] = [
                    xq_pool.tile([P, CW], F32, name=f"xt{b}_{c}", tag="xq")
                    for c in range(NCH)
                ]
                s["mt"] = [
                    m8_pool.tile([P, CW], U8, name=f"mt{b}_{c}", tag="m8")
                    for c in range(NCH)
                ]
                s["b1"] = [
                    b1_pool.tile([P, CW], F16, name=f"b1_{b}_{c}", tag="b1")
                    for c in range(NCH)
                ]
                s["accA"] = sc_pool.tile([P, 15], F32, name=f"accA_{b}", tag="accA")
                s["accB"] = sc_pool.tile([P, 10], F32, name=f"accB_{b}", tag="accB")
                s["redA"] = sc_pool.tile([P, 9], F32, name=f"redA_{b}", tag="redA")
                s["redB"] = sc_pool.tile([P, 6], F32, name=f"redB_{b}", tag="redB")
                s["stA"] = sc_pool.tile([P, 3], F32, name=f"stA_{b}", tag="stA")
                s["stB"] = sc_pool.tile([P, 2], F32, name=f"stB_{b}", tag="stB")
                s["sv"] = sc_pool.tile([P, 26], F32, name=f"sv_{b}", tag="sv")
                sv = s["sv"]
                for nmi, i in (
                    ("cntc", 0), ("inv", 1), ("mean1", 2), ("nm1", 3),
                    ("e1", 4), ("v1c", 5), ("s1", 6), ("ns1", 7),
                    ("sg1", 8), ("am1", 9), ("ncnt", 10), ("nqinv", 11),
                    ("sqc", 12), ("mean2", 13), ("nm2", 14), ("nam1", 15),
                    ("sTm", 16), ("t1", 17), ("t2", 18), ("ns1x2", 19),
                    ("sq2", 20), ("e2", 21), ("v2c", 22), ("s2", 23),
                    ("kk", 24),
                ):
                    s[nmi] = col(sv, i)

                xt, mt, accA = s["xt"], s["mt"], s["accA"]
                pa = pieces_for(b == 0)
                s["pa"] = pa
                for i, (c, o, wd) in enumerate(pa):
                    nc.sync.dma_start(
                        xt[c][:, o : o + wd],
                        x[r0 : r0 + P, c * CW + o : c * CW + o + wd],
                    )
                    nc.sync.dma_start(
                        mt[c][:, o : o + wd],
                        mk[r0 : r0 + P, c * CW + o : c * CW + o + wd],
                    )
                    # mask cast (value discarded) + cnt partial
                    cnt_t = t16_pool.tile([P, wd], F16, name=f"cnt{b}_{i}", tag="t16")
                    nc.scalar.activation(
                        cnt_t[:], mt[c][:, o : o + wd], Act.Copy,
                        accum_out=col(accA, i * 3 + 0),
                    )
                    # T = x*m (in place) + r1 partial
                    nc.vector.scalar_tensor_tensor(
                        xt[c][:, o : o + wd], xt[c][:, o : o + wd], 1.0,
                        mt[c][:, o : o + wd],
                        Alu.bypass, Alu.mult,
                        accum_out=col(accA, i * 3 + 1),
                    )
                    # r2 partial: sum(T^2); output value discarded
                    sq_t = t16_pool.tile([P, wd], F16, name=f"sq{b}_{i}", tag="t16")
                    if SQ_ENGINE[c] == "act":
                        nc.scalar.activation(
                            sq_t[:], xt[c][:, o : o + wd], Act.Square,
                            accum_out=col(accA, i * 3 + 2),
                        )
                    else:
                        nc.vector.scalar_tensor_tensor(
                            sq_t[:], xt[c][:, o : o + wd], 1.0,
                            xt[c][:, o : o + wd],
                            Alu.bypass, Alu.mult,
                            accum_out=col(accA, i * 3 + 2),
                        )

                # first-order stats; high priority so they jump both queues
                with tc.high_priority():
                    reduce_pieces(s["stA"][:, 0:3], accA, 3, len(pa), s["redA"])
                    cnt, r1, r2 = (col(s["stA"], i) for i in range(3))
                    s["cnt"], s["r1"], s["r2"] = cnt, r1, r2
                    nc.vector.tensor_scalar(s["cntc"], cnt, 1.0, None, Alu.max)
                    nc.vector.reciprocal(s["inv"], s["cntc"])
                    nc.vector.tensor_mul(s["mean1"], r1, s["inv"])
                    nc.vector.tensor_scalar(s["nm1"], s["mean1"], -1.0, None, Alu.mult)
                    nc.vector.tensor_mul(s["e1"], r2, s["inv"])
                    # v1c = max((e1 - mean1^2) * C2, EPS)
                    nc.vector.scalar_tensor_tensor(
                        s["v1c"], s["mean1"], s["nm1"], s["e1"], Alu.mult, Alu.add
                    )
                    nc.vector.tensor_scalar(
                        s["v1c"], s["v1c"], C2, EPS, Alu.mult, Alu.max
                    )
                    nc.scalar.activation(s["s1"], s["v1c"], Act.Sqrt)
                    nc.vector.tensor_scalar(s["ns1"], s["s1"], -1.0, None, Alu.mult)

            def emit_B(b):
                s = S[b]
                xt, mt, b1, accB = s["xt"], s["mt"], s["b1"], s["accB"]
                nm1, ns1 = s["nm1"], s["ns1"]
                pb = pieces_for(False)
                s["pb"] = pb
                for i, (c, o, wd) in enumerate(pb):
                    ab = ab_pool.tile([P, wd], F32, name=f"ab{b}_{i}", tag="ab")
                    if AB_ENGINE[c] == "act":
                        nc.scalar.activation(
                            ab[:], xt[c][:, o : o + wd], Act.Abs, bias=nm1,
                            accum_out=col(accB, i * 2 + 0),
                        )
                    else:
                        # |T + nm1| = abs_max(T + nm1, 0)
                        nc.vector.tensor_scalar(
                            ab[:], xt[c][:, o : o + wd], nm1, 0.0,
                            Alu.add, Alu.abs_max,
                            accum_out=col(accB, i * 2 + 0),
                        )
                    nc.scalar.activation(
                        b1[c][:, o : o + wd], xt[c][:, o : o + wd],
                        Act.Sign, bias=nm1,
                    )
                    # q = (ab - s1) * b1, in place onto the T tile
                    nc.vector.scalar_tensor_tensor(
                        xt[c][:, o : o + wd], ab[:], ns1, b1[c][:, o : o + wd],
                        Alu.add, Alu.mult,
                        accum_out=col(accB, i * 2 + 1),
                    )

                # correction scalars (low priority; only needed at the reduce)
                nc.scalar.activation(s["sg1"], s["mean1"], Act.Sign)
                nc.vector.tensor_mul(s["am1"], s["mean1"], s["sg1"])
                nc.vector.tensor_scalar(
                    s["ncnt"], s["cnt"], -1.0, float(N), Alu.mult, Alu.add
                )
                nc.vector.scalar_tensor_tensor(
                    s["nqinv"], s["sg1"], s["ns1"], s["mean1"], Alu.mult, Alu.add
                )
                nc.vector.tensor_scalar(s["nam1"], s["am1"], -1.0, None, Alu.mult)

                # second-order stats
                with tc.high_priority():
                    reduce_pieces(s["stB"][:, 0:2], accB, 2, len(pb), s["redB"])
                    sab, sq = col(s["stB"], 0), col(s["stB"], 1)
                    nc.vector.scalar_tensor_tensor(
                        s["sqc"], s["ncnt"], s["nqinv"], sq, Alu.mult, Alu.add
                    )
                    nc.vector.tensor_mul(s["mean2"], s["sqc"], s["inv"])
                    nc.vector.tensor_scalar(
                        s["nm2"], s["mean2"], -1.0, None, Alu.mult
                    )
                    nc.vector.scalar_tensor_tensor(
                        s["sTm"], s["ncnt"], s["nam1"], sab, Alu.mult, Alu.add
                    )
                    nc.vector.scalar_tensor_tensor(
                        s["t1"], s["r1"], s["nm1"], s["r2"], Alu.mult, Alu.add
                    )
                    nc.vector.scalar_tensor_tensor(
                        s["t2"], s["cnt"], s["v1c"], s["t1"], Alu.mult, Alu.add
                    )
                    nc.vector.tensor_scalar(s["ns1x2"], s["s1"], -2.0, None, Alu.mult)
                    nc.vector.scalar_tensor_tensor(
                        s["sq2"], s["sTm"], s["ns1x2"], s["t2"], Alu.mult, Alu.add
                    )
                    nc.vector.tensor_mul(s["e2"], s["sq2"], s["inv"])
                    nc.vector.scalar_tensor_tensor(
                        s["v2c"], s["mean2"], s["nm2"], s["e2"], Alu.mult, Alu.add
                    )
                    nc.vector.tensor_scalar(
                        s["v2c"], s["v2c"], C2, EPS, Alu.mult, Alu.max
                    )
                    nc.scalar.activation(s["s2"], s["v2c"], Act.Sqrt)
                    nc.vector.tensor_add(s["kk"], s["mean1"], s["mean2"])

            def emit_C(b):
                s = S[b]
                r0 = b * P
                xt, mt, b1 = s["xt"], s["mt"], s["b1"]
                nm2, s1, s2, kk = s["nm2"], s["s1"], s["s2"], s["kk"]
                u_on_act = False
                for i, (c, o, wd) in enumerate(pieces_for(False)):
                    b2 = b2_pool.tile([P, wd], F16, name=f"b2_{b}_{i}", tag="b2")
                    nc.scalar.activation(
                        b2[:], xt[c][:, o : o + wd], Act.Sign, bias=nm2
                    )
                    # u = s2*b2 + K (TS 4x, in place)
                    if u_on_act:
                        nc.scalar.activation(
                            b2[:], b2[:], Act.Identity, bias=kk, scale=s2
                        )
                    else:
                        nc.vector.tensor_scalar(
                            b2[:], b2[:], s2, kk, Alu.mult, Alu.add
                        )
                    # bs1 = s1*b1 (TS 4x, in place)
                    nc.vector.tensor_scalar(
                        b1[c][:, o : o + wd], b1[c][:, o : o + wd], s1, None,
                        Alu.mult,
                    )
                    # w = u + bs1 (TT 2x, in place)
                    nc.vector.tensor_add(b2[:], b2[:], b1[c][:, o : o + wd])
                    # out = w * m -> f32, overwrites the q tile
                    nc.vector.tensor_mul(
                        xt[c][:, o : o + wd], b2[:], mt[c][:, o : o + wd]
                    )
                    nc.sync.dma_start(
                        out[r0 : r0 + P, c * CW + o : c * CW + o + wd],
                        xt[c][:, o : o + wd],
                    )

            # software-pipelined emission: next-block A lands ahead of the
            # previous block's C in every engine queue
            emit_A(0)
            emit_B(0)
            for b in range(1, NBLK):
                emit_A(b)
                emit_C(b - 1)
                emit_B(b)
            emit_C(NBLK - 1)

    return nc


def get_program():
    if "nc" not in _CACHE:
        nc = _build_program()
        nc.finalize()
        _CACHE["nc"] = nc
    return _CACHE["nc"]


def kernel(x: np.ndarray, mask: np.ndarray) -> np.ndarray:
    import time

    from concourse.bass_utils import run_bass_kernel_spmd

    x = np.ascontiguousarray(np.asarray(x, dtype=np.float32))
    mask = np.ascontiguousarray(np.asarray(mask))
    if mask.dtype == np.bool_ or mask.dtype == np.uint8:
        mask_u8 = mask.view(np.uint8)
    else:
        mask_u8 = (mask != 0).astype(np.uint8)
    assert x.shape == (R * NCORES, N), x.shape
    assert mask_u8.shape == (R * NCORES, N), mask_u8.shape

    nc = get_program()
    in_maps = [
        {
            "x": x[k * R : (k + 1) * R],
            "mask": mask_u8[k * R : (k + 1) * R],
        }
        for k in range(NCORES)
    ]
    last_err = None
    for attempt in range(3):
        try:
            res = run_bass_kernel_spmd(nc, in_maps, core_ids=list(range(NCORES)))
            return np.concatenate([r["out"] for r in res.results], axis=0)
        except Exception as e:  # transient NRT/device hiccups
            last_err = e
            if attempt < 2:
                time.sleep(10)
    raise last_err


if __name__ == "__main__":
    xs = np.random.randn(R * NCORES, N).astype(np.float32)
    ms = (np.random.randint(0, 2, (R * NCORES, N))).astype(bool)
    y = kernel(xs, ms)
    print(y.shape, y.dtype)


# revision 30
# speedup vs baseline: 1.0491x; 1.0189x over previous
"""Trainium2 Bass kernel for adaptive_high_order_residual_v2 (ORDER=2 masked
sign-binarization, per-row stats).

Full-input contract: kernel(x, mask) takes the complete (4096, 11008) arrays,
shards rows across 8 NeuronCores (512 rows each; per-row reductions make this
embarrassingly parallel), runs one SPMD Bass program, and concatenates the
per-core outputs.

Math per row (exact restructuring of the reference, ORDER = 2):
  T     = x*m                                   (masked input; in-place on x)
  cnt   = sum(m); r1 = sum(T); r2 = sum(T^2)
  mean1 = r1/cnt ; var1 = r2/cnt - mean1^2 ; s1 = sqrt(var1 * 2/pi)
  ab    = |T - mean1|  (+ accum; invalid entries contribute |mean1| each,
                        corrected by scalar algebra)
  b1    = sign(T - mean1)      (unmasked: invalid entries give sign(-mean1))
  q     = (ab - s1) * b1       (+ accum sum q; invalid garbage is the
                                per-row constant s1*sign(mean1)-mean1,
                                corrected by scalar algebra)
  sum q^2 = (r2 - mean1*r1) + cnt*s1^2 - 2*s1*sum|T - mean1|_masked
            (no elementwise pass needed)
  mean2, s2 from the corrected sums; K = mean1 + mean2
  out   = (K + s1*b1 + s2*b2) * m,  b2 = sign(q - mean2)

Engine split (measured per-chunk-of-2752 on HW):
  ACT : mask cast + cnt, sum(T^2) (3/4 chunks), abs+accum, sign1, sign2
  DVE : T=x*m+r1 (STT), sum(T^2) (1/4), q (STT), u=s2*b2+K (TS 4x),
        bs1=s1*b1 (TS 4x), w=u+bs1 (TT 2x), out=w*m (TT)
  Pool: unused - concurrent GPSIMD activity degrades DVE 2x/4x modes ~2.5x
        (SBUF port contention), a net loss every time it was tried.

Emission is software-pipelined (A(0) B(0) A(1) C(0) B(1) A(2) C(1) ...) so
the tile list-scheduler interleaves next-block stage-A work into the C-stage
stream; per-block scalar reductions run under high_priority so they jump both
engine queues.
"""

import sys

import numpy as np

sys.path.insert(0, "/opt/trn_rl_repo")

R = 512          # rows per core
N = 11008        # columns
P = 128          # SBUF partitions per row-block
NBLK = R // P    # 4 blocks per core
CW = 2752        # column chunk width
NCH = N // CW    # 4 chunks per block
NCORES = 8
C2 = 0.6366197723675814  # 2/pi
EPS = 1e-30

# Per-chunk engine assignment for the flexible accumulation passes
# ("act" or "dve"); index is chunk id within a block.
SQ_ENGINE = ["act", "act", "act", "dve"]       # sum(T^2) accum
AB_ENGINE = ["act"] * NCH                      # |T - mean1| + accum

_CACHE = {}


def _build_program():
    import concourse.bacc as bacc
    import concourse.mybir as mybir
    from concourse.tile import TileContext

    F32 = mybir.dt.float32
    F16 = mybir.dt.float16
    U8 = mybir.dt.uint8
    Alu = mybir.AluOpType
    Act = mybir.ActivationFunctionType

    nc = bacc.Bacc()
    x = nc.dram_tensor("x", [R, N], F32, kind="ExternalInput")
    mk = nc.dram_tensor("mask", [R, N], U8, kind="ExternalInput")
    out = nc.dram_tensor("out", [R, N], F32, kind="ExternalOutput")

    H = CW // 2

    def pieces_for(split_first):
        ps = []
        for c in range(NCH):
            if c == 0 and split_first:
                ps.append((c, 0, H))
                ps.append((c, H, H))
            else:
                ps.append((c, 0, CW))
        return ps

    with TileContext(nc) as tc:
        with (
            tc.tile_pool(name="xq", bufs=2 * NCH + 1) as xq_pool,   # x -> T -> q -> out
            tc.tile_pool(name="m8", bufs=2 * NCH) as m8_pool,   # u8 mask
            tc.tile_pool(name="b1", bufs=2 * NCH - 1) as b1_pool,   # sign1 -> s1*b1 (f16)
            tc.tile_pool(name="ab", bufs=2) as ab_pool,         # |T - mean1| (f32)
            tc.tile_pool(name="b2", bufs=3) as b2_pool,         # sign2 -> u -> w (f16)
            tc.tile_pool(name="t16", bufs=2) as t16_pool,       # discard outputs (f16)
            tc.tile_pool(name="sc", bufs=2) as sc_pool,         # accums + scalars
        ):
            S = [dict() for _ in range(NBLK)]

            def col(t, i):
                return t[:, i : i + 1]

            def reduce_pieces(dst, acc, k, npieces, red):
                # piece-major accum columns: col = p*k + q
                if npieces == 4:
                    nc.vector.tensor_add(
                        red[:, 0 : 2 * k], acc[:, 0 : 2 * k], acc[:, 2 * k : 4 * k]
                    )
                    nc.vector.tensor_add(dst, red[:, 0:k], red[:, k : 2 * k])
                elif npieces == 5:
                    nc.vector.tensor_add(
                        red[:, 0 : 2 * k], acc[:, 0 : 2 * k], acc[:, 2 * k : 4 * k]
                    )
                    nc.vector.tensor_add(
                        red[:, 2 * k : 3 * k], red[:, 0:k], red[:, k : 2 * k]
                    )
                    nc.vector.tensor_add(
                        dst, red[:, 2 * k : 3 * k], acc[:, 4 * k : 5 * k]
                    )
                else:
                    raise AssertionError(npieces)

            def emit_A(b):
                s = S[b]
                r0 = b * P
                s["xt"] = [
                    xq_pool.tile([P, CW], F32, name=f"xt{b}_{c}", tag="xq")
                    for c in range(NCH)
                ]
                s["mt"] = [
                    m8_pool.tile([P, CW], U8, name=f"mt{b}_{c}", tag="m8")
                    for c in range(NCH)
                ]
                s["b1"] = [
                    b1_pool.tile([P, CW], F16, name=f"b1_{b}_{c}", tag="b1")
                    for c in range(NCH)
                ]
                s["accA"] = sc_pool.tile([P, 15], F32, name=f"accA_{b}", tag="accA")
                s["accB"] = sc_pool.tile([P, 10], F32, name=f"accB_{b}", tag="accB")
                s["redA"] = sc_pool.tile([P, 9], F32, name=f"redA_{b}", tag="redA")
                s["redB"] = sc_pool.tile([P, 6], F32, name=f"redB_{b}", tag="redB")
                s["stA"] = sc_pool.tile([P, 3], F32, name=f"stA_{b}", tag="stA")
                s["stB"] = sc_pool.tile([P, 2], F32, name=f"stB_{b}", tag="stB")
                s["sv"] = sc_pool.tile([P, 26], F32, name=f"sv_{b}", tag="sv")
                sv = s["sv"]
                for nmi, i in (
                    ("cntc", 0), ("inv", 1), ("mean1", 2), ("nm1", 3),
                    ("e1", 4), ("v1c", 5), ("s1", 6), ("ns1", 7),
                    ("sg1", 8), ("am1", 9), ("ncnt", 10), ("nqinv", 11),
                    ("sqc", 12), ("mean2", 13), ("nm2", 14), ("nam1", 15),
                    ("sTm", 16), ("t1", 17), ("t2", 18), ("ns1x2", 19),
                    ("sq2", 20), ("e2", 21), ("v2c", 22), ("s2", 23),
                    ("kk", 24),
                ):
                    s[nmi] = col(sv, i)

                xt, mt, accA = s["xt"], s["mt"], s["accA"]
                pa = pieces_for(b == 0)
                s["pa"] = pa
                for i, (c, o, wd) in enumerate(pa):
                    nc.sync.dma_start(
                        xt[c][:, o : o + wd],
                        x[r0 : r0 + P, c * CW + o : c * CW + o + wd],
                    )
                    nc.sync.dma_start(
                        mt[c][:, o : o + wd],
                        mk[r0 : r0 + P, c * CW + o : c * CW + o + wd],
                    )
                    # mask cast (value discarded) + cnt partial
                    cnt_t = t16_pool.tile([P, wd], F16, name=f"cnt{b}_{i}", tag="t16")
                    nc.scalar.activation(
                        cnt_t[:], mt[c][:, o : o + wd], Act.Copy,
                        accum_out=col(accA, i * 3 + 0),
                    )
                    # T = x*m (in place) + r1 partial
                    nc.vector.scalar_tensor_tensor(
                        xt[c][:, o : o + wd], xt[c][:, o : o + wd], 1.0,
                        mt[c][:, o : o + wd],
                        Alu.bypass, Alu.mult,
                        accum_out=col(accA, i * 3 + 1),
                    )
                    # r2 partial: sum(T^2); output value discarded
                    sq_t = t16_pool.tile([P, wd], F16, name=f"sq{b}_{i}", tag="t16")
                    if SQ_ENGINE[c] == "act":
                        nc.scalar.activation(
                            sq_t[:], xt[c][:, o : o + wd], Act.Square,
                            accum_out=col(accA, i * 3 + 2),
                        )
                    else:
                        nc.vector.scalar_tensor_tensor(
                            sq_t[:], xt[c][:, o : o + wd], 1.0,
                            xt[c][:, o : o + wd],
                            Alu.bypass, Alu.mult,
                            accum_out=col(accA, i * 3 + 2),
                        )

                # first-order stats; high priority so they jump both queues
                with tc.high_priority():
                    reduce_pieces(s["stA"][:, 0:3], accA, 3, len(pa), s["redA"])
                    cnt, r1, r2 = (col(s["stA"], i) for i in range(3))
                    s["cnt"], s["r1"], s["r2"] = cnt, r1, r2
                    nc.vector.tensor_scalar(s["cntc"], cnt, 1.0, None, Alu.max)
                    nc.vector.reciprocal(s["inv"], s["cntc"])
                    nc.vector.tensor_mul(s["mean1"], r1, s["inv"])
                    nc.vector.tensor_scalar(s["nm1"], s["mean1"], -1.0, None, Alu.mult)
                    nc.vector.tensor_mul(s["e1"], r2, s["inv"])
                    # v1c = max((e1 - mean1^2) * C2, EPS)
                    nc.vector.scalar_tensor_tensor(
                        s["v1c"], s["mean1"], s["nm1"], s["e1"], Alu.mult, Alu.add
                    )
                    nc.vector.tensor_scalar(
                        s["v1c"], s["v1c"], C2, EPS, Alu.mult, Alu.max
                    )
                    nc.scalar.activation(s["s1"], s["v1c"], Act.Sqrt)
                    nc.vector.tensor_scalar(s["ns1"], s["s1"], -1.0, None, Alu.mult)

            def emit_B(b):
                s = S[b]
                xt, mt, b1, accB = s["xt"], s["mt"], s["b1"], s["accB"]
                nm1, ns1 = s["nm1"], s["ns1"]
                pb = pieces_for(True)
                s["pb"] = pb
                for i, (c, o, wd) in enumerate(pb):
                    ab = ab_pool.tile([P, wd], F32, name=f"ab{b}_{i}", tag="ab")
                    if AB_ENGINE[c] == "act":
                        nc.scalar.activation(
                            ab[:], xt[c][:, o : o + wd], Act.Abs, bias=nm1,
                            accum_out=col(accB, i * 2 + 0),
                        )
                    else:
                        # |T + nm1| = abs_max(T + nm1, 0)
                        nc.vector.tensor_scalar(
                            ab[:], xt[c][:, o : o + wd], nm1, 0.0,
                            Alu.add, Alu.abs_max,
                            accum_out=col(accB, i * 2 + 0),
                        )
                    nc.scalar.activation(
                        b1[c][:, o : o + wd], xt[c][:, o : o + wd],
                        Act.Sign, bias=nm1,
                    )
                    # q = (ab - s1) * b1, in place onto the T tile
                    nc.vector.scalar_tensor_tensor(
                        xt[c][:, o : o + wd], ab[:], ns1, b1[c][:, o : o + wd],
                        Alu.add, Alu.mult,
                        accum_out=col(accB, i * 2 + 1),
                    )

                # correction scalars (low priority; only needed at the reduce)
                nc.scalar.activation(s["sg1"], s["mean1"], Act.Sign)
                nc.vector.tensor_mul(s["am1"], s["mean1"], s["sg1"])
                nc.vector.tensor_scalar(
                    s["ncnt"], s["cnt"], -1.0, float(N), Alu.mult, Alu.add
                )
                nc.vector.scalar_tensor_tensor(
                    s["nqinv"], s["sg1"], s["ns1"], s["mean1"], Alu.mult, Alu.add
                )
                nc.vector.tensor_scalar(s["nam1"], s["am1"], -1.0, None, Alu.mult)

                # second-order stats
                with tc.high_priority():
                    reduce_pieces(s["stB"][:, 0:2], accB, 2, len(pb), s["redB"])
                    sab, sq = col(s["stB"], 0), col(s["stB"], 1)
                    nc.vector.scalar_tensor_tensor(
                        s["sqc"], s["ncnt"], s["nqinv"], sq, Alu.mult, Alu.add
                    )
                    nc.vector.tensor_mul(s["mean2"], s["sqc"], s["inv"])
                    nc.vector.tensor_scalar(
                        s["nm2"], s["mean2"], -1.0, None, Alu.mult
                    )
                    nc.vector.scalar_tensor_tensor(
                        s["sTm"], s["ncnt"], s["nam1"], sab, Alu.mult, Alu.add
                    )
                    nc.vector.scalar_tensor_tensor(
                        s["t1"], s["r1"], s["nm1"], s["r2"], Alu.mult, Alu.add
                    )
                    nc.vector.scalar_tensor_tensor(
                        s["t2"], s["cnt"], s["v1c"], s["t1"], Alu.mult, Alu.add
                    )
                    nc.vector.tensor_scalar(s["ns1x2"], s["s1"], -2.0, None, Alu.mult)
                    nc.vector.scalar_tensor_tensor(
                        s["sq2"], s["sTm"], s["ns1x2"], s["t2"], Alu.mult, Alu.add
                    )
                    nc.vector.tensor_mul(s["e2"], s["sq2"], s["inv"])
                    nc.vector.scalar_tensor_tensor(
                        s["v2c"], s["mean2"], s["nm2"], s["e2"], Alu.mult, Alu.add
                    )
                    nc.vector.tensor_scalar(
                        s["v2c"], s["v2c"], C2, EPS, Alu.mult, Alu.max
                    )
                    nc.scalar.activation(s["s2"], s["v2c"], Act.Sqrt)
                    nc.vector.tensor_add(s["kk"], s["mean1"], s["mean2"])

            def emit_C(b):
                s = S[b]
                r0 = b * P
                xt, mt, b1 = s["xt"], s["mt"], s["b1"]
                nm2, s1, s2, kk = s["nm2"], s["s1"], s["s2"], s["kk"]
                u_on_act = False
                for i, (c, o, wd) in enumerate(pieces_for(True)):
                    b2 = b2_pool.tile([P, wd], F16, name=f"b2_{b}_{i}", tag="b2")
                    nc.scalar.activation(
                        b2[:], xt[c][:, o : o + wd], Act.Sign, bias=nm2
                    )
                    # u = s2*b2 + K (TS 4x, in place)
                    if u_on_act:
                        nc.scalar.activation(
                            b2[:], b2[:], Act.Identity, bias=kk, scale=s2
                        )
                    else:
                        nc.vector.tensor_scalar(
                            b2[:], b2[:], s2, kk, Alu.mult, Alu.add
                        )
                    # bs1 = s1*b1 (TS 4x, in place)
                    nc.vector.tensor_scalar(
                        b1[c][:, o : o + wd], b1[c][:, o : o + wd], s1, None,
                        Alu.mult,
                    )
                    # w = u + bs1 (TT 2x, in place)
                    nc.vector.tensor_add(b2[:], b2[:], b1[c][:, o : o + wd])
                    # out = w * m -> f32, overwrites the q tile
                    nc.vector.tensor_mul(
                        xt[c][:, o : o + wd], b2[:], mt[c][:, o : o + wd]
                    )
                    nc.sync.dma_start(
                        out[r0 : r0 + P, c * CW + o : c * CW + o + wd],
                        xt[c][:, o : o + wd],
                    )

            # software-pipelined emission: next-block A lands ahead of the
            # previous block's C in every engine queue
            emit_A(0)
            emit_B(0)
            for b in range(1, NBLK):
                emit_A(b)
                emit_C(b - 1)
                emit_B(b)
            emit_C(NBLK - 1)

    return nc


def get_program():
    if "nc" not in _CACHE:
        nc = _build_program()
        nc.finalize()
        _CACHE["nc"] = nc
    return _CACHE["nc"]


def kernel(x: np.ndarray, mask: np.ndarray) -> np.ndarray:
    import time

    from concourse.bass_utils import run_bass_kernel_spmd

    x = np.ascontiguousarray(np.asarray(x, dtype=np.float32))
    mask = np.ascontiguousarray(np.asarray(mask))
    if mask.dtype == np.bool_ or mask.dtype == np.uint8:
        mask_u8 = mask.view(np.uint8)
    else:
        mask_u8 = (mask != 0).astype(np.uint8)
    assert x.shape == (R * NCORES, N), x.shape
    assert mask_u8.shape == (R * NCORES, N), mask_u8.shape

    nc = get_program()
    in_maps = [
        {
            "x": x[k * R : (k + 1) * R],
            "mask": mask_u8[k * R : (k + 1) * R],
        }
        for k in range(NCORES)
    ]
    last_err = None
    for attempt in range(3):
        try:
            res = run_bass_kernel_spmd(nc, in_maps, core_ids=list(range(NCORES)))
            return np.concatenate([r["out"] for r in res.results], axis=0)
        except Exception as e:  # transient NRT/device hiccups
            last_err = e
            if attempt < 2:
                time.sleep(10)
    raise last_err


if __name__ == "__main__":
    xs = np.random.randn(R * NCORES, N).astype(np.float32)
    ms = (np.random.randint(0, 2, (R * NCORES, N))).astype(bool)
    y = kernel(xs, ms)
    print(y.shape, y.dtype)


# revision 32
# speedup vs baseline: 1.0538x; 1.0046x over previous
"""Trainium2 Bass kernel for adaptive_high_order_residual_v2 (ORDER=2 masked
sign-binarization, per-row stats).

Full-input contract: kernel(x, mask) takes the complete (4096, 11008) arrays,
shards rows across 8 NeuronCores (512 rows each; per-row reductions make this
embarrassingly parallel), runs one SPMD Bass program, and concatenates the
per-core outputs.

Math per row (exact restructuring of the reference, ORDER = 2):
  T     = x*m                                   (masked input; in-place on x)
  cnt   = sum(m); r1 = sum(T); r2 = sum(T^2)
  mean1 = r1/cnt ; var1 = r2/cnt - mean1^2 ; s1 = sqrt(var1 * 2/pi)
  ab    = |T - mean1|  (+ accum; invalid entries contribute |mean1| each,
                        corrected by scalar algebra)
  b1    = sign(T - mean1)      (unmasked: invalid entries give sign(-mean1))
  q     = (ab - s1) * b1       (+ accum sum q; invalid garbage is the
                                per-row constant s1*sign(mean1)-mean1,
                                corrected by scalar algebra)
  sum q^2 = (r2 - mean1*r1) + cnt*s1^2 - 2*s1*sum|T - mean1|_masked
            (no elementwise pass needed)
  mean2, s2 from the corrected sums; K = mean1 + mean2
  out   = (K + s1*b1 + s2*b2) * m,  b2 = sign(q - mean2)

Engine split (measured per-chunk-of-2752 on HW):
  ACT : mask cast + cnt, sum(T^2) (3/4 chunks), abs+accum, sign1, sign2
  DVE : T=x*m+r1 (STT), sum(T^2) (1/4), q (STT), u=s2*b2+K (TS 4x),
        bs1=s1*b1 (TS 4x), w=u+bs1 (TT 2x), out=w*m (TT)
  Pool: unused - concurrent GPSIMD activity degrades DVE 2x/4x modes ~2.5x
        (SBUF port contention), a net loss every time it was tried.

Emission is software-pipelined (A(0) B(0) A(1) C(0) B(1) A(2) C(1) ...) so
the tile list-scheduler interleaves next-block stage-A work into the C-stage
stream; per-block scalar reductions run under high_priority so they jump both
engine queues.
"""

import sys

import numpy as np

sys.path.insert(0, "/opt/trn_rl_repo")

R = 512          # rows per core
N = 11008        # columns
P = 128          # SBUF partitions per row-block
NBLK = R // P    # 4 blocks per core
CW = 2752        # column chunk width
NCH = N // CW    # 4 chunks per block
NCORES = 8
C2 = 0.6366197723675814  # 2/pi
EPS = 1e-30

# Per-chunk engine assignment for the flexible accumulation passes
# ("act" or "dve"); index is chunk id within a block.
SQ_ENGINE = ["act", "act", "act", "dve"]       # sum(T^2) accum
AB_ENGINE = ["act"] * NCH                      # |T - mean1| + accum

_CACHE = {}


def _build_program():
    import concourse.bacc as bacc
    import concourse.mybir as mybir
    from concourse.tile import TileContext

    F32 = mybir.dt.float32
    F16 = mybir.dt.float16
    U8 = mybir.dt.uint8
    Alu = mybir.AluOpType
    Act = mybir.ActivationFunctionType

    nc = bacc.Bacc()
    x = nc.dram_tensor("x", [R, N], F32, kind="ExternalInput")
    mk = nc.dram_tensor("mask", [R, N], U8, kind="ExternalInput")
    out = nc.dram_tensor("out", [R, N], F32, kind="ExternalOutput")

    H = CW // 2

    def pieces_for(split_first, split_all=False):
        ps = []
        for c in range(NCH):
            if split_all or (c == 0 and split_first):
                ps.append((c, 0, H))
                ps.append((c, H, H))
            else:
                ps.append((c, 0, CW))
        return ps

    with TileContext(nc) as tc:
        with (
            tc.tile_pool(name="xq", bufs=2 * NCH + 1) as xq_pool,   # x -> T -> q -> out
            tc.tile_pool(name="m8", bufs=2 * NCH) as m8_pool,   # u8 mask
            tc.tile_pool(name="b1", bufs=2 * NCH - 1) as b1_pool,   # sign1 -> s1*b1 (f16)
            tc.tile_pool(name="ab", bufs=2) as ab_pool,         # |T - mean1| (f32)
            tc.tile_pool(name="b2", bufs=3) as b2_pool,         # sign2 -> u -> w (f16)
            tc.tile_pool(name="t16", bufs=2) as t16_pool,       # discard outputs (f16)
            tc.tile_pool(name="sc", bufs=2) as sc_pool,         # accums + scalars
        ):
            S = [dict() for _ in range(NBLK)]

            def col(t, i):
                return t[:, i : i + 1]

            def reduce_pieces(dst, acc, k, npieces, red):
                # piece-major accum columns: col = p*k + q
                if npieces == 4:
                    nc.vector.tensor_add(
                        red[:, 0 : 2 * k], acc[:, 0 : 2 * k], acc[:, 2 * k : 4 * k]
                    )
                    nc.vector.tensor_add(dst, red[:, 0:k], red[:, k : 2 * k])
                elif npieces == 5:
                    nc.vector.tensor_add(
                        red[:, 0 : 2 * k], acc[:, 0 : 2 * k], acc[:, 2 * k : 4 * k]
                    )
                    nc.vector.tensor_add(
                        red[:, 2 * k : 3 * k], red[:, 0:k], red[:, k : 2 * k]
                    )
                    nc.vector.tensor_add(
                        dst, red[:, 2 * k : 3 * k], acc[:, 4 * k : 5 * k]
                    )
                else:
                    raise AssertionError(npieces)

            def emit_A(b):
                s = S[b]
                r0 = b * P
                s["xt"] = [
                    xq_pool.tile([P, CW], F32, name=f"xt{b}_{c}", tag="xq")
                    for c in range(NCH)
                ]
                s["mt"] = [
                    m8_pool.tile([P, CW], U8, name=f"mt{b}_{c}", tag="m8")
                    for c in range(NCH)
                ]
                s["b1"] = [
                    b1_pool.tile([P, CW], F16, name=f"b1_{b}_{c}", tag="b1")
                    for c in range(NCH)
                ]
                s["accA"] = sc_pool.tile([P, 15], F32, name=f"accA_{b}", tag="accA")
                s["accB"] = sc_pool.tile([P, 10], F32, name=f"accB_{b}", tag="accB")
                s["redA"] = sc_pool.tile([P, 9], F32, name=f"redA_{b}", tag="redA")
                s["redB"] = sc_pool.tile([P, 6], F32, name=f"redB_{b}", tag="redB")
                s["stA"] = sc_pool.tile([P, 3], F32, name=f"stA_{b}", tag="stA")
                s["stB"] = sc_pool.tile([P, 2], F32, name=f"stB_{b}", tag="stB")
                s["sv"] = sc_pool.tile([P, 26], F32, name=f"sv_{b}", tag="sv")
                sv = s["sv"]
                for nmi, i in (
                    ("cntc", 0), ("inv", 1), ("mean1", 2), ("nm1", 3),
                    ("e1", 4), ("v1c", 5), ("s1", 6), ("ns1", 7),
                    ("sg1", 8), ("am1", 9), ("ncnt", 10), ("nqinv", 11),
                    ("sqc", 12), ("mean2", 13), ("nm2", 14), ("nam1", 15),
                    ("sTm", 16), ("t1", 17), ("t2", 18), ("ns1x2", 19),
                    ("sq2", 20), ("e2", 21), ("v2c", 22), ("s2", 23),
                    ("kk", 24),
                ):
                    s[nmi] = col(sv, i)

                xt, mt, accA = s["xt"], s["mt"], s["accA"]
                pa = pieces_for(b == 0)
                s["pa"] = pa
                for i, (c, o, wd) in enumerate(pa):
                    nc.sync.dma_start(
                        xt[c][:, o : o + wd],
                        x[r0 : r0 + P, c * CW + o : c * CW + o + wd],
                    )
                    nc.sync.dma_start(
                        mt[c][:, o : o + wd],
                        mk[r0 : r0 + P, c * CW + o : c * CW + o + wd],
                    )
                    # mask cast (value discarded) + cnt partial
                    cnt_t = t16_pool.tile([P, wd], F16, name=f"cnt{b}_{i}", tag="t16")
                    nc.scalar.activation(
                        cnt_t[:], mt[c][:, o : o + wd], Act.Copy,
                        accum_out=col(accA, i * 3 + 0),
                    )
                    # T = x*m (in place) + r1 partial
                    nc.vector.scalar_tensor_tensor(
                        xt[c][:, o : o + wd], xt[c][:, o : o + wd], 1.0,
                        mt[c][:, o : o + wd],
                        Alu.bypass, Alu.mult,
                        accum_out=col(accA, i * 3 + 1),
                    )
                    # r2 partial: sum(T^2); output value discarded
                    sq_t = t16_pool.tile([P, wd], F16, name=f"sq{b}_{i}", tag="t16")
                    if SQ_ENGINE[c] == "act":
                        nc.scalar.activation(
                            sq_t[:], xt[c][:, o : o + wd], Act.Square,
                            accum_out=col(accA, i * 3 + 2),
                        )
                    else:
                        nc.vector.scalar_tensor_tensor(
                            sq_t[:], xt[c][:, o : o + wd], 1.0,
                            xt[c][:, o : o + wd],
                            Alu.bypass, Alu.mult,
                            accum_out=col(accA, i * 3 + 2),
                        )

                # first-order stats; high priority so they jump both queues
                with tc.high_priority():
                    reduce_pieces(s["stA"][:, 0:3], accA, 3, len(pa), s["redA"])
                    cnt, r1, r2 = (col(s["stA"], i) for i in range(3))
                    s["cnt"], s["r1"], s["r2"] = cnt, r1, r2
                    nc.vector.tensor_scalar(s["cntc"], cnt, 1.0, None, Alu.max)
                    nc.vector.reciprocal(s["inv"], s["cntc"])
                    nc.vector.tensor_mul(s["mean1"], r1, s["inv"])
                    nc.vector.tensor_scalar(s["nm1"], s["mean1"], -1.0, None, Alu.mult)
                    nc.vector.tensor_mul(s["e1"], r2, s["inv"])
                    # v1c = max((e1 - mean1^2) * C2, EPS)
                    nc.vector.scalar_tensor_tensor(
                        s["v1c"], s["mean1"], s["nm1"], s["e1"], Alu.mult, Alu.add
                    )
                    nc.vector.tensor_scalar(
                        s["v1c"], s["v1c"], C2, EPS, Alu.mult, Alu.max
                    )
                    nc.scalar.activation(s["s1"], s["v1c"], Act.Sqrt)
                    nc.vector.tensor_scalar(s["ns1"], s["s1"], -1.0, None, Alu.mult)

            def emit_B(b):
                s = S[b]
                xt, mt, b1, accB = s["xt"], s["mt"], s["b1"], s["accB"]
                nm1, ns1 = s["nm1"], s["ns1"]
                pb = pieces_for(True)
                s["pb"] = pb
                for i, (c, o, wd) in enumerate(pb):
                    ab = ab_pool.tile([P, wd], F32, name=f"ab{b}_{i}", tag="ab")
                    if AB_ENGINE[c] == "act":
                        nc.scalar.activation(
                            ab[:], xt[c][:, o : o + wd], Act.Abs, bias=nm1,
                            accum_out=col(accB, i * 2 + 0),
                        )
                    else:
                        # |T + nm1| = abs_max(T + nm1, 0)
                        nc.vector.tensor_scalar(
                            ab[:], xt[c][:, o : o + wd], nm1, 0.0,
                            Alu.add, Alu.abs_max,
                            accum_out=col(accB, i * 2 + 0),
                        )
                    nc.scalar.activation(
                        b1[c][:, o : o + wd], xt[c][:, o : o + wd],
                        Act.Sign, bias=nm1,
                    )
                    # q = (ab - s1) * b1, in place onto the T tile
                    nc.vector.scalar_tensor_tensor(
                        xt[c][:, o : o + wd], ab[:], ns1, b1[c][:, o : o + wd],
                        Alu.add, Alu.mult,
                        accum_out=col(accB, i * 2 + 1),
                    )

                # correction scalars (low priority; only needed at the reduce)
                nc.scalar.activation(s["sg1"], s["mean1"], Act.Sign)
                nc.vector.tensor_mul(s["am1"], s["mean1"], s["sg1"])
                nc.vector.tensor_scalar(
                    s["ncnt"], s["cnt"], -1.0, float(N), Alu.mult, Alu.add
                )
                nc.vector.scalar_tensor_tensor(
                    s["nqinv"], s["sg1"], s["ns1"], s["mean1"], Alu.mult, Alu.add
                )
                nc.vector.tensor_scalar(s["nam1"], s["am1"], -1.0, None, Alu.mult)

                # second-order stats
                with tc.high_priority():
                    reduce_pieces(s["stB"][:, 0:2], accB, 2, len(pb), s["redB"])
                    sab, sq = col(s["stB"], 0), col(s["stB"], 1)
                    nc.vector.scalar_tensor_tensor(
                        s["sqc"], s["ncnt"], s["nqinv"], sq, Alu.mult, Alu.add
                    )
                    nc.vector.tensor_mul(s["mean2"], s["sqc"], s["inv"])
                    nc.vector.tensor_scalar(
                        s["nm2"], s["mean2"], -1.0, None, Alu.mult
                    )
                    nc.vector.scalar_tensor_tensor(
                        s["sTm"], s["ncnt"], s["nam1"], sab, Alu.mult, Alu.add
                    )
                    nc.vector.scalar_tensor_tensor(
                        s["t1"], s["r1"], s["nm1"], s["r2"], Alu.mult, Alu.add
                    )
                    nc.vector.scalar_tensor_tensor(
                        s["t2"], s["cnt"], s["v1c"], s["t1"], Alu.mult, Alu.add
                    )
                    nc.vector.tensor_scalar(s["ns1x2"], s["s1"], -2.0, None, Alu.mult)
                    nc.vector.scalar_tensor_tensor(
                        s["sq2"], s["sTm"], s["ns1x2"], s["t2"], Alu.mult, Alu.add
                    )
                    nc.vector.tensor_mul(s["e2"], s["sq2"], s["inv"])
                    nc.vector.scalar_tensor_tensor(
                        s["v2c"], s["mean2"], s["nm2"], s["e2"], Alu.mult, Alu.add
                    )
                    nc.vector.tensor_scalar(
                        s["v2c"], s["v2c"], C2, EPS, Alu.mult, Alu.max
                    )
                    nc.scalar.activation(s["s2"], s["v2c"], Act.Sqrt)
                    nc.vector.tensor_add(s["kk"], s["mean1"], s["mean2"])

            def emit_C(b):
                s = S[b]
                r0 = b * P
                xt, mt, b1 = s["xt"], s["mt"], s["b1"]
                nm2, s1, s2, kk = s["nm2"], s["s1"], s["s2"], s["kk"]
                u_on_act = False
                for i, (c, o, wd) in enumerate(
                    pieces_for(True, split_all=(b == NBLK - 1))
                ):
                    b2 = b2_pool.tile([P, wd], F16, name=f"b2_{b}_{i}", tag="b2")
                    nc.scalar.activation(
                        b2[:], xt[c][:, o : o + wd], Act.Sign, bias=nm2
                    )
                    # u = s2*b2 + K (TS 4x, in place)
                    if u_on_act:
                        nc.scalar.activation(
                            b2[:], b2[:], Act.Identity, bias=kk, scale=s2
                        )
                    else:
                        nc.vector.tensor_scalar(
                            b2[:], b2[:], s2, kk, Alu.mult, Alu.add
                        )
                    # bs1 = s1*b1 (TS 4x, in place)
                    nc.vector.tensor_scalar(
                        b1[c][:, o : o + wd], b1[c][:, o : o + wd], s1, None,
                        Alu.mult,
                    )
                    # w = u + bs1 (TT 2x, in place)
                    nc.vector.tensor_add(b2[:], b2[:], b1[c][:, o : o + wd])
                    # out = w * m -> f32, overwrites the q tile
                    nc.vector.tensor_mul(
                        xt[c][:, o : o + wd], b2[:], mt[c][:, o : o + wd]
                    )
                    nc.sync.dma_start(
                        out[r0 : r0 + P, c * CW + o : c * CW + o + wd],
                        xt[c][:, o : o + wd],
                    )

            # software-pipelined emission: next-block A lands ahead of the
            # previous block's C in every engine queue
            emit_A(0)
            emit_B(0)
            for b in range(1, NBLK):
                emit_A(b)
                emit_C(b - 1)
                emit_B(b)
            emit_C(NBLK - 1)

    return nc


def get_program():
    if "nc" not in _CACHE:
        nc = _build_program()
        nc.finalize()
        _CACHE["nc"] = nc
    return _CACHE["nc"]


def kernel(x: np.ndarray, mask: np.ndarray) -> np.ndarray:
    import time

    from concourse.bass_utils import run_bass_kernel_spmd

    x = np.ascontiguousarray(np.asarray(x, dtype=np.float32))
    mask = np.ascontiguousarray(np.asarray(mask))
    if mask.dtype == np.bool_ or mask.dtype == np.uint8:
        mask_u8 = mask.view(np.uint8)
    else:
        mask_u8 = (mask != 0).astype(np.uint8)
    assert x.shape == (R * NCORES, N), x.shape
    assert mask_u8.shape == (R * NCORES, N), mask_u8.shape

    nc = get_program()
    in_maps = [
        {
            "x": x[k * R : (k + 1) * R],
            "mask": mask_u8[k * R : (k + 1) * R],
        }
        for k in range(NCORES)
    ]
    last_err = None
    for attempt in range(3):
        try:
            res = run_bass_kernel_spmd(nc, in_maps, core_ids=list(range(NCORES)))
            return np.concatenate([r["out"] for r in res.results], axis=0)
        except Exception as e:  # transient NRT/device hiccups
            last_err = e
            if attempt < 2:
                time.sleep(10)
    raise last_err


if __name__ == "__main__":
    xs = np.random.randn(R * NCORES, N).astype(np.float32)
    ms = (np.random.randint(0, 2, (R * NCORES, N))).astype(bool)
    y = kernel(xs, ms)
    print(y.shape, y.dtype)
